# revision 2
# baseline (speedup 1.0000x reference)
"""Free-axis Trainium kernel for the coupled-pendulum ring ODE.

Pure data parallel over 8 cores: batch 1024 -> 128 rows/core = SBUF
partitions; ring 512 = free axis, so the ring Laplacian is shifted-view
elementwise ops (plus two [128,1] wrap-column ops) — no PE matmuls.

Integrator: 3-stage Runge-Kutta-Nystrom order 4 in rescaled time
tau = omega0*t, angles in turns (theta_hat = theta/2pi), V = h*u:
    B1'=(k1-part), B2', C3' unscaled combines  B' = d + n/(2pi*cp),
    d = lap(x), n = -sin(2pi x); true B_i = c_i*B'_i with c1 = cp*h^2/6,
    c2 = cp*h^2/3, c3 = c1 folded into downstream immediates.
    p2 = th + V/2 + 0.75 B1;  p3 = th + V + 1.5 B2
    th' = th + V + B1 + B2;   V' = V + B1 + 2 B2 + C3
96 steps gives rel err ~4.5e-3 vs the float64 odeint reference
(validated against it in numpy and on hardware).

sin args are range-reduced to [-0.5, 0.5] turns on DVE via the
1.5*2^23 round trick (the ACT Sin table is only valid on ~[-pi, pi];
measured: garbage beyond).
"""

import math

import numpy as np

import concourse.bacc as bacc
import concourse.bass as bass
import concourse.dve_ops as dve_ops
import concourse.mybir as mybir
import concourse.tile as tile
from concourse.bass_utils import run_bass_kernel_spmd
from concourse.dve_spec import C0, C1, C2, Spec, Src0, Src1, _has_src1, lower
from concourse.dve_uop import DveOpSpec

F32 = mybir.dt.float32
AF = mybir.ActivationFunctionType
OP = mybir.AluOpType

N_CORES = 8
B, N = 1024, 512
PB = B // N_CORES

NSTEPS = 96
T_END = 2.0
TWO_PI = 2 * math.pi
MAGIC = 12582912.0

USE_WRAP = True
NCHUNK = 1
OUTER_REPS = 1
WRAP_POOL = set()

# Pool (gpsimd) only supports plain tensor_tensor on TRN2 — every
# scaled op (scalar_tensor_tensor) must go to DVE.  Pool gets the
# unscaled adds; everything else is DVE.
ENGINE_OF = {
    "s1": "P", "d1": "V", "B1": "V",
    "s2": "P", "d2": "V", "B2": "V",
    "s3": "P", "d3": "V", "B3": "V",
    "p2": "V", "p3": "V", "gc": "P", "thn": "V",
    "z1": "V", "z2": "P", "vn": "V", "q": "V", "t": "P",
    "sb": "P", "w1p": "P", "w2p": "P", "w3p": "P",
}


def _register_custom_op(name, body, reference):
    for op in dve_ops.OPS:
        if op.name == name:
            return op
    idx = dve_ops._CUSTOM_DVE_ROW_BASE + len(dve_ops.OPS)
    assert idx < 0x20
    spec = Spec(body=body, reference=reference)
    shas = {}
    for ver in ("v3", "v4"):
        try:
            uops = lower(spec, ver=ver)
            tmp = DveOpSpec(name=name, opcode=idx, uops=uops,
                            rd1_en=_has_src1(spec))
            shas[ver] = tmp.sha(ver)
        except Exception:
            pass
    op = dve_ops.DveOp(name, spec, subdim=False, uops_sha=shas)
    dve_ops.OPS.append(op)
    dve_ops._SUB_OPCODE_FOR_NAME[name] = idx
    dve_ops.CUSTOM_DVE_SPECS[name] = spec
    return op


def _f32(v):
    return np.float32(v)


_tw_z = Src0 * C0 + Src1 * C1
TURNS_WRAP = _register_custom_op(
    "TURNS_WRAP_ANT",
    _tw_z - ((_tw_z + C2) - C2),
    lambda in0, in1, s0, s1, imm2: (
        lambda z: z - ((z + _f32(imm2)) - _f32(imm2)))(
        (in0.astype(np.float32) * _f32(s0)
         + in1.astype(np.float32) * _f32(s1)).astype(np.float32)),
)


def _build(nsteps: int, omega0: float, coupling: float,
           outer_reps: int = None) -> bass.Bass:
    if outer_reps is None:
        outer_reps = OUTER_REPS
    tau_end = omega0 * T_END
    h = tau_end / nsteps
    cp = coupling / (omega0 * omega0)
    hh6 = h * h / 6.0
    hh3 = h * h / 3.0

    W = N // NCHUNK
    chunks = [(c * W, (c + 1) * W) for c in range(NCHUNK)]
    schunks = [(max(a, 1), min(b, N - 1)) for (a, b) in chunks]

    nc = bacc.Bacc("TRN2", target_bir_lowering=False, debug=False,
                   num_devices=N_CORES)
    x_in = nc.dram_tensor("x", [PB, N], F32, kind="ExternalInput")
    out = nc.dram_tensor("out", [PB, N], F32, kind="ExternalOutput")

    with tile.TileContext(nc) as tc:
        with (
            tc.tile_pool(name="state", bufs=1) as state,
            tc.tile_pool(name="tmp", bufs=3) as tmp,
        ):
            ths = [state.tile([128, N], F32, name=f"th{i}", tag=f"th{i}")
                   for i in range(2)]
            vs = [state.tile([128, N], F32, name=f"v{i}", tag=f"v{i}")
                  for i in range(2)]
            nc.vector.memset(vs[0][:], 0.0)

            xstage = state.tile([128, N], F32, name="xstage", tag="xstage")
            nc.gpsimd.dma_start(xstage[:], x_in[:])
            nc.scalar.activation(ths[0][:, :], xstage[:], AF.Copy,
                                 bias=-0.5, scale=1.0)

            def eng(key):
                return {"V": nc.vector, "P": nc.gpsimd,
                        "A": nc.scalar}[ENGINE_OF[key]]

            def stt(key, outap, in0, scalar, in1, op1=OP.add):
                e = eng(key)
                if e is nc.gpsimd:
                    # Pool has no scalar_tensor_tensor on TRN2
                    assert scalar == 1.0 and op1 == OP.add, key
                    e.tensor_add(outap, in0, in1)
                else:
                    e.scalar_tensor_tensor(outap, in0, float(scalar), in1,
                                           OP.mult, op1)

            def tile_s(nm):
                return tmp.tile([128, N], F32, name=nm, tag=nm)

            prev = {}
            kn = 1.0 / (TWO_PI * cp)
            c1 = cp * hh6
            c2 = cp * hh3
            c3 = cp * hh6

            def eval_force(x, skey, Bout, wrap_args):
                sT = tile_s("s" + skey)
                dT = tile_s("d" + skey)
                nT = tile_s("n" + skey)
                wT = tile_s("w" + skey) if USE_WRAP else None
                rT = tile_s("r" + skey) if skey in WRAP_POOL else None
                stt("sb", sT[:, 0:1], x[:, N - 1:N], 1.0, x[:, 1:2])
                stt("sb", sT[:, N - 1:N], x[:, N - 2:N - 1], 1.0, x[:, 0:1])
                for (a, b) in schunks:
                    stt("s" + skey, sT[:, a:b], x[:, a - 1:b - 1], 1.0,
                        x[:, a + 1:b + 1])
                for (a, b) in chunks:
                    if not USE_WRAP:
                        nsrc = x[:, a:b]
                    elif skey in WRAP_POOL:
                        nc.gpsimd.tensor_scalar(rT[:, a:b], x[:, a:b],
                                                MAGIC, MAGIC, OP.add,
                                                OP.subtract)
                        stt("w" + skey + "p", wT[:, a:b], rT[:, a:b], -1.0,
                            x[:, a:b])
                        nsrc = wT[:, a:b]
                    else:
                        t0, t1, s0, s1 = wrap_args
                        nc.vector._custom_dve(
                            TURNS_WRAP, out=wT[:, a:b], in0=t0[:, a:b],
                            in1=t1[:, a:b], s0=float(s0), s1=float(s1),
                            imm2=MAGIC)
                        nsrc = wT[:, a:b]
                    nc.scalar.activation(nT[:, a:b], nsrc, AF.Sin,
                                         scale=-TWO_PI)
                    if skey in ("1", "3"):
                        # produce B/2: d = s/2 - x, combine with kn/2
                        stt("d" + skey, dT[:, a:b], sT[:, a:b], 0.5,
                            x[:, a:b], OP.subtract)
                        stt("B" + skey, Bout[:, a:b], nT[:, a:b], 0.5 * kn,
                            dT[:, a:b])
                    else:
                        stt("d" + skey, dT[:, a:b], x[:, a:b], -2.0,
                            sT[:, a:b])
                        stt("B" + skey, Bout[:, a:b], nT[:, a:b], kn,
                            dT[:, a:b])
                return Bout

            def step(th, th_new, v, v_new, first):
                B1 = tile_s("B1")
                B2 = tile_s("B2")
                C3 = tile_s("C3")
                p2 = tile_s("p2")
                p3 = tile_s("p3")
                gcT = tile_s("gc")
                qn = tile_s("qn")
                tn = tile_s("tn")
                z1 = tile_s("z1")
                z2 = tile_s("z2")

                if first:
                    q = tile_s("qn")
                    t = tile_s("tn")
                    nc.scalar.copy(q[:], th[:])
                    nc.scalar.copy(t[:], th[:])
                else:
                    q, t = prev["q"], prev["t"]
                wrap1 = (th, xstage, 1.0, 0.0)

                eval_force(th, "1", B1, wrap1)
                for (a, b) in chunks:
                    stt("p2", p2[:, a:b], B1[:, a:b], 1.5 * c1, q[:, a:b])
                eval_force(p2, "2", B2, (B1, q, 1.5 * c1, 1.0))
                for (a, b) in chunks:
                    stt("gc", gcT[:, a:b], B1[:, a:b], 1.0, B2[:, a:b])
                    stt("thn", th_new[:, a:b], gcT[:, a:b], c2, t[:, a:b])
                    stt("z1", z1[:, a:b], gcT[:, a:b], c2, v[:, a:b])
                    stt("p3", p3[:, a:b], B2[:, a:b], 1.5 * c2, t[:, a:b])
                eval_force(p3, "3", C3, (B2, t, 1.5 * c2, 1.0))
                for (a, b) in chunks:
                    stt("z2", z2[:, a:b], B2[:, a:b], 1.0, C3[:, a:b])
                    stt("vn", v_new[:, a:b], z2[:, a:b], 2.0 * c3,
                        z1[:, a:b])
                    stt("q", qn[:, a:b], v_new[:, a:b], 0.5,
                        th_new[:, a:b])
                    stt("t", tn[:, a:b], v_new[:, a:b], 1.0,
                        th_new[:, a:b])

                prev.update(q=qn, t=tn)

            def run_integration(first):
                for i in range(nsteps):
                    step(ths[i % 2], ths[(i + 1) % 2], vs[i % 2],
                         vs[(i + 1) % 2], first and i == 0)

            if outer_reps == 1:
                run_integration(True)
            else:
                run_integration(True)
                with tc.For_i(0, outer_reps - 1) as _:
                    run_integration(False)

            fin = ths[nsteps % 2]
            rad = tile_s("rad")
            nc.scalar.activation(rad[:], fin[:], AF.Copy,
                                 bias=0.0, scale=TWO_PI)
            nc.gpsimd.dma_start(out[:], rad[:])

    nc.compile()
    return nc


_CACHE: dict = {}


def kernel(x, omega0, coupling, nsteps: int = None):
    x = np.ascontiguousarray(np.asarray(x, dtype=np.float32))
    om = float(np.asarray(omega0, dtype=np.float64))
    cpl = float(np.asarray(coupling, dtype=np.float64))
    if nsteps is None:
        nsteps = NSTEPS
    key = (nsteps, om, cpl)
    if key not in _CACHE:
        _CACHE[key] = _build(nsteps, om, cpl)
    nc = _CACHE[key]

    in_maps = [{"x": x[i * PB:(i + 1) * PB]} for i in range(N_CORES)]
    res = run_bass_kernel_spmd(nc, in_maps, list(range(N_CORES)))
    return np.concatenate([r["out"] for r in res.results],
                          axis=0).astype(np.float32)
